# revision 14
# baseline (speedup 1.0000x reference)
"""Trainium2 Bass kernel for nn_Exchange (topk channel exchange).

y1 = x1 with its non-top-|bn1| channels replaced by x2's non-top-|bn2|
channels (order-aligned), y2 symmetric.  The op is a pure row
permutation of [x1; x2] onto [y1; y2]: every input channel row lands in
exactly one output row.

Sharding: batch dim (B=8) across 8 cores, one [C, L] slice per core.
bn1/bn2 and the topk/mask/index computation are replicated on every core.

Per-core schedule (scatter formulation — hides the index-computation
latency behind the input loads, which have no data dependency):
  1. 8 contiguous HWDGE loads stage all of x1/x2 into SBUF, starting
     immediately.
  2. Meanwhile the engines compute, from bn1/bn2 alone, the destination
     row of every input channel (top-k by |bn| via pairwise rank,
     prefix sums via scan, non-top position matching via is_equal).
  3. 16 indirect SWDGE scatters write each 128-row SBUF chunk to its
     destination rows; rows belonging to the other output are marked
     out-of-bounds and skipped (bounds_check), so each chunk is
     scattered once into y1 and once into y2.
"""

import sys

for _p in ("/opt/trn_rl_repo", "/opt/pypackages"):
    if _p not in sys.path:
        sys.path.append(_p)

from contextlib import ExitStack

import numpy as np

import concourse.bass as bass
import concourse.tile as tile
from concourse import bacc, mybir
from concourse.bass_utils import run_bass_kernel_spmd

F32 = mybir.dt.float32
I32 = mybir.dt.int32
U8 = mybir.dt.uint8
OP = mybir.AluOpType

B, C, L = 8, 512, 4096
K = 256  # topk = C * (1 - EXCHANGE_RATIO)
P = 128
NCH = C // P  # 4 chunks of 128 channels
N_CORES = 8
OOB = 600.0  # > C-1 bounds_check -> row skipped (small: offset*4096 must fit i32)

TRACE = False
LAST_RESULTS = None


def _emit(tc):
    nc = tc.nc
    x1 = nc.dram_tensor("x1", [C, L], F32, kind="ExternalInput").ap()
    x2 = nc.dram_tensor("x2", [C, L], F32, kind="ExternalInput").ap()
    bn1 = nc.dram_tensor("bn1", [C], F32, kind="ExternalInput").ap()
    bn2 = nc.dram_tensor("bn2", [C], F32, kind="ExternalInput").ap()
    y1 = nc.dram_tensor("y1", [C, L], F32, kind="ExternalOutput").ap()
    y2 = nc.dram_tensor("y2", [C, L], F32, kind="ExternalOutput").ap()

    with ExitStack() as ctx:
        const = ctx.enter_context(tc.tile_pool(name="const", bufs=1))
        small = ctx.enter_context(tc.tile_pool(name="small", bufs=1))
        psum = ctx.enter_context(tc.tile_pool(name="psum", bufs=1, space="PSUM"))
        bulk = ctx.enter_context(tc.tile_pool(name="bulk", bufs=8))

        # ---- tiny bn loads first (ahead of the bulk loads on the same
        # HWDGE queue), then the 8 bulk input loads — no data deps, so
        # they stream from t=0 while the index math runs.
        a_raw1 = small.tile([1, C], F32)
        nc.sync.dma_start(out=a_raw1[:], in_=bn1[None, :])
        a_raw2 = small.tile([1, C], F32)
        nc.sync.dma_start(out=a_raw2[:], in_=bn2[None, :])

        xt1 = []
        xt2 = []
        for k in range(NCH):
            t = bulk.tile([P, L], F32, name=f"xt1_{k}", tag="xt")
            nc.sync.dma_start(out=t[:], in_=x1[k * P : (k + 1) * P, :])
            xt1.append(t)
        for k in range(NCH):
            t = bulk.tile([P, L], F32, name=f"xt2_{k}", tag="xt")
            nc.sync.dma_start(out=t[:], in_=x2[k * P : (k + 1) * P, :])
            xt2.append(t)

        # ---- constants ----
        ones_row = const.tile([1, P], F32)
        nc.vector.memset(ones_row[:], 1.0)
        ones_col = const.tile([P, 1], F32)
        nc.vector.memset(ones_col[:], 1.0)
        zeros_row = const.tile([1, C], F32)
        nc.vector.memset(zeros_row[:], 0.0)
        big_row = const.tile([1, C], F32)
        nc.vector.memset(big_row[:], 9999.0)
        oob_col = const.tile([P, NCH], F32)
        nc.vector.memset(oob_col[:], OOB)
        # jrow_f[p, j] = j  for all partitions
        jrow_i = const.tile([P, C], I32)
        nc.gpsimd.iota(jrow_i[:], pattern=[[1, C]], base=0, channel_multiplier=0)
        jrow_f = const.tile([P, C], F32)
        nc.vector.tensor_copy(jrow_f[:], jrow_i[:])
        # iota_col_f[p, i] = i*128 + p  (channel index in column layout)
        iota_col_i = const.tile([P, NCH], I32)
        nc.gpsimd.iota(iota_col_i[:], pattern=[[P, NCH]], base=0, channel_multiplier=1)
        iota_col_f = const.tile([P, NCH], F32)
        nc.vector.tensor_copy(iota_col_f[:], iota_col_i[:])

        def bn_stats(a_raw, tag):
            """Per-bn stats from the raw [1, C] bn row.

            Returns:
              z_col_m [P, NCH] u8:  1 where channel is NOT in topk of |bn|
              px_col  [P, NCH] f32: exclusive prefix count of non-top before c
              pm_row_b[P, C]  f32: same prefix in row layout broadcast along
                                   partitions, 9999.0 on top channels
            """
            # |bn| row: (raw * -1) max raw
            a_row = small.tile([1, C], F32, name=f"a_row_{tag}")
            nc.vector.scalar_tensor_tensor(
                out=a_row[:], in0=a_raw[:], scalar=-1.0, in1=a_raw[:],
                op0=OP.mult, op1=OP.max,
            )
            # broadcast to all partitions: arow_b[p, j] = |bn[j]|
            ab_ps = psum.tile([P, C], F32, name=f"ab_ps_{tag}", tag=f"ps_ab_{tag}")
            nc.tensor.matmul(
                out=ab_ps[:], lhsT=ones_row[:], rhs=a_row[:], start=True, stop=True
            )
            arow_b = small.tile([P, C], F32, name=f"arow_b_{tag}")
            nc.vector.tensor_copy(arow_b[:], ab_ps[:])
            # column layout: acol[p, i] = |bn[i*128+p]|
            acol_ps = psum.tile(
                [P, NCH], F32, name=f"acol_ps_{tag}", tag=f"ps_acol_{tag}"
            )
            for i in range(NCH):
                nc.tensor.matmul(
                    out=acol_ps[:, i : i + 1],
                    lhsT=a_row[0:1, i * P : (i + 1) * P],
                    rhs=ones_row[0:1, 0:1],
                    start=True,
                    stop=True,
                )
            acol = small.tile([P, NCH], F32, name=f"acol_{tag}")
            nc.vector.tensor_copy(acol[:], acol_ps[:])

            # rank[c] = #{j : |bn[j]| > |bn[c]|}; G_i materialized for the
            # partition-sum matmul, rank_col via fused free-dim accumulate
            rank_col = small.tile([P, NCH], F32, name=f"rank_col_{tag}")
            rank_ps = psum.tile([1, C], F32, name=f"rank_ps_{tag}", tag="ps_rank")
            gs = []
            for i in range(NCH):
                g = small.tile([P, C], F32, name=f"G_{tag}_{i}")
                nc.vector.tensor_scalar(
                    out=g[:],
                    in0=arow_b[:],
                    scalar1=acol[:, i : i + 1],
                    scalar2=None,
                    op0=OP.is_gt,
                    op1=OP.add,
                    accum_out=rank_col[:, i : i + 1],
                )
                gs.append(g)
            for i in range(NCH):
                nc.tensor.matmul(
                    out=rank_ps[:],
                    lhsT=ones_col[:],
                    rhs=gs[i][:],
                    start=(i == 0),
                    stop=(i == NCH - 1),
                )
            # colsum of G gives #{i : a[i] < a[j]}; rank[j] = (C-1) - colsum
            # (values assumed distinct, as in the reference's random normals)
            rank_row = small.tile([1, C], F32, name=f"rank_row_{tag}")
            nc.vector.tensor_scalar(
                out=rank_row[:], in0=rank_ps[:], scalar1=-1.0,
                scalar2=float(C - 1), op0=OP.mult, op1=OP.add,
            )

            # non-top masks (rank >= K); u8 copies because CopyPredicated
            # requires an integer mask
            z_row = small.tile([1, C], F32, name=f"z_row_{tag}")
            nc.vector.tensor_scalar(
                out=z_row[:], in0=rank_row[:], scalar1=K - 0.5, scalar2=None,
                op0=OP.is_gt,
            )
            z_row_m = small.tile([1, C], U8, name=f"z_row_m_{tag}")
            nc.vector.tensor_scalar(
                out=z_row_m[:], in0=rank_row[:], scalar1=K - 0.5, scalar2=None,
                op0=OP.is_gt,
            )
            z_col_m = small.tile([P, NCH], U8, name=f"z_col_m_{tag}")
            nc.vector.tensor_scalar(
                out=z_col_m[:], in0=rank_col[:], scalar1=K - 0.5, scalar2=None,
                op0=OP.is_gt,
            )

            # exclusive prefix sum of z along channel order
            pincl_row = small.tile([1, C], F32, name=f"pincl_row_{tag}")
            nc.vector.tensor_tensor_scan(
                out=pincl_row[:], data0=z_row[:], data1=zeros_row[:], initial=0.0,
                op0=OP.add, op1=OP.add,
            )
            pexcl_row = small.tile([1, C], F32, name=f"pexcl_row_{tag}")
            nc.vector.tensor_tensor(
                out=pexcl_row[:], in0=pincl_row[:], in1=z_row[:], op=OP.subtract
            )

            # masked prefix row (9999 on top channels), broadcast to partitions
            pm_row = small.tile([1, C], F32, name=f"pm_row_{tag}")
            nc.vector.select(
                out=pm_row[:], mask=z_row_m[:], on_true=pexcl_row[:],
                on_false=big_row[:],
            )
            pm_ps = psum.tile([P, C], F32, name=f"pm_ps_{tag}", tag="ps_pm")
            nc.tensor.matmul(
                out=pm_ps[:], lhsT=ones_row[:], rhs=pm_row[:], start=True, stop=True
            )
            pm_row_b = small.tile([P, C], F32, name=f"pm_row_b_{tag}")
            nc.vector.tensor_copy(pm_row_b[:], pm_ps[:])

            # prefix in column layout
            px_ps = psum.tile([P, NCH], F32, name=f"px_ps_{tag}", tag=f"ps_px_{tag}")
            for i in range(NCH):
                nc.tensor.matmul(
                    out=px_ps[:, i : i + 1],
                    lhsT=pexcl_row[0:1, i * P : (i + 1) * P],
                    rhs=ones_row[0:1, 0:1],
                    start=True,
                    stop=True,
                )
            px_col = small.tile([P, NCH], F32, name=f"px_col_{tag}")
            nc.vector.tensor_copy(px_col[:], px_ps[:])

            return z_col_m, px_col, pm_row_b

        z1_col, px1_col, pm1_row_b = bn_stats(a_raw1, "b1")
        z2_col, px2_col, pm2_row_b = bn_stats(a_raw2, "b2")

        def keep_dest(z_col, tag):
            """dest row for kept channels: c if in topk else OOB-skip."""
            df = small.tile([P, NCH], F32, name=f"dfk_{tag}")
            nc.vector.select(
                out=df[:], mask=z_col[:], on_true=oob_col[:], on_false=iota_col_f[:]
            )
            d = small.tile([P, NCH], I32, name=f"dk_{tag}")
            nc.vector.tensor_copy(d[:], df[:])
            return d

        def exch_dest(z_col, px_col, other_pm_row_b, tag):
            """dest row in the OTHER output for non-top channels:
            nt_other[px[c]] (matched via is_equal against the masked
            other-side prefix row), OOB-skip for kept channels."""
            srcx_col = small.tile([P, NCH], F32, name=f"srcx_{tag}")
            for i in range(NCH):
                mt = small.tile([P, C], F32, name=f"mt_{tag}_{i}", tag="mt", bufs=2)
                nc.vector.scalar_tensor_tensor(
                    out=mt[:],
                    in0=other_pm_row_b[:],
                    scalar=px_col[:, i : i + 1],
                    in1=jrow_f[:],
                    op0=OP.is_equal,
                    op1=OP.mult,
                    accum_out=srcx_col[:, i : i + 1],
                )
            df = small.tile([P, NCH], F32, name=f"dfe_{tag}")
            nc.vector.select(
                out=df[:], mask=z_col[:], on_true=srcx_col[:], on_false=oob_col[:]
            )
            d = small.tile([P, NCH], I32, name=f"de_{tag}")
            nc.vector.tensor_copy(d[:], df[:])
            return d

        d_x1_y1 = keep_dest(z1_col, "x1y1")
        d_x2_y2 = keep_dest(z2_col, "x2y2")
        d_x1_y2 = exch_dest(z1_col, px1_col, pm2_row_b, "x1y2")
        d_x2_y1 = exch_dest(z2_col, px2_col, pm1_row_b, "x2y1")

        # ---- scatters: each input chunk goes once to y1 and once to y2,
        # with the rows belonging to the other output OOB-skipped.
        # Keep-scatters first (their dest tables are ready earliest);
        # alternate output tensors so same-tensor write ordering overlaps.
        def scat(y, d, k, src):
            nc.gpsimd.indirect_dma_start(
                out=y[:, :],
                out_offset=bass.IndirectOffsetOnAxis(ap=d[:, k : k + 1], axis=0),
                in_=src[:],
                in_offset=None,
                bounds_check=C - 1,
                oob_is_err=False,
            )

        for k in range(NCH):
            scat(y1, d_x1_y1, k, xt1[k])
            scat(y2, d_x2_y2, k, xt2[k])
        for k in range(NCH):
            scat(y2, d_x1_y2, k, xt1[k])
            scat(y1, d_x2_y1, k, xt2[k])


def build_nc(compile=True):
    nc = bacc.Bacc(
        "TRN2",
        target_bir_lowering=False,
        debug=False,
        enable_asserts=False,
        num_devices=N_CORES,
    )
    with tile.TileContext(nc) as tc:
        _emit(tc)
    if compile:
        nc.compile()
    return nc


_NC = None


def _get_nc():
    global _NC
    if _NC is None:
        _NC = build_nc()
    return _NC


def kernel(x1, x2, bn1, bn2):
    global LAST_RESULTS
    x1 = np.ascontiguousarray(np.asarray(x1), dtype=np.float32)
    x2 = np.ascontiguousarray(np.asarray(x2), dtype=np.float32)
    bn1 = np.ascontiguousarray(np.asarray(bn1), dtype=np.float32)
    bn2 = np.ascontiguousarray(np.asarray(bn2), dtype=np.float32)
    assert x1.shape == (B, C, L) and x2.shape == (B, C, L)

    nc = _get_nc()
    in_maps = [
        {"x1": x1[i], "x2": x2[i], "bn1": bn1, "bn2": bn2}
        for i in range(N_CORES)
    ]
    res = run_bass_kernel_spmd(
        nc, in_maps, core_ids=list(range(N_CORES)), trace=TRACE
    )
    LAST_RESULTS = res
    y1 = np.stack([r["y1"] for r in res.results], axis=0)
    y2 = np.stack([r["y2"] for r in res.results], axis=0)
    return (y1, y2)


# revision 15
# speedup vs baseline: 1.0320x; 1.0320x over previous
"""Trainium2 Bass kernel for nn_Exchange (topk channel exchange).

y1 = x1 with its non-top-|bn1| channels replaced by x2's non-top-|bn2|
channels (order-aligned), y2 symmetric.  The op is a pure row
permutation of [x1; x2] onto [y1; y2]: every input channel row lands in
exactly one output row.

Sharding: batch dim (B=8) across 8 cores, one [C, L] slice per core.
bn1/bn2 and the topk/mask/index computation are replicated on every core.

Per-core schedule (scatter formulation — hides the index-computation
latency behind the input loads, which have no data dependency):
  1. 8 contiguous HWDGE loads stage all of x1/x2 into SBUF, starting
     immediately.
  2. Meanwhile the engines compute, from bn1/bn2 alone, the destination
     row of every input channel (top-k by |bn| via pairwise rank,
     prefix sums via scan, non-top position matching via is_equal).
  3. 16 indirect SWDGE scatters write each 128-row SBUF chunk to its
     destination rows; rows belonging to the other output are marked
     out-of-bounds and skipped (bounds_check), so each chunk is
     scattered once into y1 and once into y2.
"""

import sys

for _p in ("/opt/trn_rl_repo", "/opt/pypackages"):
    if _p not in sys.path:
        sys.path.append(_p)

from contextlib import ExitStack

import numpy as np

import concourse.bass as bass
import concourse.tile as tile
from concourse import bacc, mybir
from concourse.bass_utils import run_bass_kernel_spmd

F32 = mybir.dt.float32
I32 = mybir.dt.int32
U8 = mybir.dt.uint8
OP = mybir.AluOpType

B, C, L = 8, 512, 4096
K = 256  # topk = C * (1 - EXCHANGE_RATIO)
P = 128
NCH = C // P  # 4 chunks of 128 channels
N_CORES = 8
OOB = 600.0  # > C-1 bounds_check -> row skipped (small: offset*4096 must fit i32)

TRACE = False
LAST_RESULTS = None


def _emit(tc):
    nc = tc.nc
    x1 = nc.dram_tensor("x1", [C, L], F32, kind="ExternalInput").ap()
    x2 = nc.dram_tensor("x2", [C, L], F32, kind="ExternalInput").ap()
    bn1 = nc.dram_tensor("bn1", [C], F32, kind="ExternalInput").ap()
    bn2 = nc.dram_tensor("bn2", [C], F32, kind="ExternalInput").ap()
    y1 = nc.dram_tensor("y1", [C, L], F32, kind="ExternalOutput").ap()
    y2 = nc.dram_tensor("y2", [C, L], F32, kind="ExternalOutput").ap()

    with ExitStack() as ctx:
        const = ctx.enter_context(tc.tile_pool(name="const", bufs=1))
        small = ctx.enter_context(tc.tile_pool(name="small", bufs=1))
        psum = ctx.enter_context(tc.tile_pool(name="psum", bufs=1, space="PSUM"))
        bulk = ctx.enter_context(tc.tile_pool(name="bulk", bufs=8))

        # ---- tiny bn loads first (ahead of the bulk loads on the same
        # HWDGE queue), then the 8 bulk input loads — no data deps, so
        # they stream from t=0 while the index math runs.
        a_raw1 = small.tile([1, C], F32)
        nc.sync.dma_start(out=a_raw1[:], in_=bn1[None, :])
        a_raw2 = small.tile([1, C], F32)
        nc.sync.dma_start(out=a_raw2[:], in_=bn2[None, :])

        xt1 = []
        xt2 = []
        for k in range(NCH):
            t = bulk.tile([P, L], F32, name=f"xt1_{k}", tag="xt")
            nc.sync.dma_start(out=t[:], in_=x1[k * P : (k + 1) * P, :])
            xt1.append(t)
        for k in range(NCH):
            t = bulk.tile([P, L], F32, name=f"xt2_{k}", tag="xt")
            nc.sync.dma_start(out=t[:], in_=x2[k * P : (k + 1) * P, :])
            xt2.append(t)

        # ---- constants ----
        ones_row = const.tile([1, P], F32)
        nc.vector.memset(ones_row[:], 1.0)
        ones_col = const.tile([P, 1], F32)
        nc.vector.memset(ones_col[:], 1.0)
        zeros_row = const.tile([1, C], F32)
        nc.vector.memset(zeros_row[:], 0.0)
        big_row = const.tile([1, C], F32)
        nc.vector.memset(big_row[:], 9999.0)
        oob_col = const.tile([P, NCH], F32)
        nc.vector.memset(oob_col[:], OOB)
        # jrow_f[p, j] = j  for all partitions
        jrow_i = const.tile([P, C], I32)
        nc.gpsimd.iota(jrow_i[:], pattern=[[1, C]], base=0, channel_multiplier=0)
        jrow_f = const.tile([P, C], F32)
        nc.vector.tensor_copy(jrow_f[:], jrow_i[:])
        # iota_col_f[p, i] = i*128 + p  (channel index in column layout)
        iota_col_i = const.tile([P, NCH], I32)
        nc.gpsimd.iota(iota_col_i[:], pattern=[[P, NCH]], base=0, channel_multiplier=1)
        iota_col_f = const.tile([P, NCH], F32)
        nc.vector.tensor_copy(iota_col_f[:], iota_col_i[:])

        def bn_stats(a_raw, tag):
            """Per-bn stats from the raw [1, C] bn row.

            Returns:
              z_col_m [P, NCH] u8:  1 where channel is NOT in topk of |bn|
              px_col  [P, NCH] f32: exclusive prefix count of non-top before c
              pm_row_b[P, C]  f32: same prefix in row layout broadcast along
                                   partitions, 9999.0 on top channels
            """
            # |bn| row: (raw * -1) max raw
            a_row = small.tile([1, C], F32, name=f"a_row_{tag}")
            nc.vector.scalar_tensor_tensor(
                out=a_row[:], in0=a_raw[:], scalar=-1.0, in1=a_raw[:],
                op0=OP.mult, op1=OP.max,
            )
            # broadcast to all partitions: arow_b[p, j] = |bn[j]|
            ab_ps = psum.tile([P, C], F32, name=f"ab_ps_{tag}", tag=f"ps_ab_{tag}")
            nc.tensor.matmul(
                out=ab_ps[:], lhsT=ones_row[:], rhs=a_row[:], start=True, stop=True
            )
            arow_b = small.tile([P, C], F32, name=f"arow_b_{tag}")
            nc.vector.tensor_copy(arow_b[:], ab_ps[:])
            # column layout: acol[p, i] = |bn[i*128+p]|
            acol_ps = psum.tile(
                [P, NCH], F32, name=f"acol_ps_{tag}", tag=f"ps_acol_{tag}"
            )
            for i in range(NCH):
                nc.tensor.matmul(
                    out=acol_ps[:, i : i + 1],
                    lhsT=a_row[0:1, i * P : (i + 1) * P],
                    rhs=ones_row[0:1, 0:1],
                    start=True,
                    stop=True,
                )
            acol = small.tile([P, NCH], F32, name=f"acol_{tag}")
            nc.vector.tensor_copy(acol[:], acol_ps[:])

            # rank[c] = #{j : |bn[j]| > |bn[c]|}; G_i materialized for the
            # partition-sum matmul, rank_col via fused free-dim accumulate
            rank_col = small.tile([P, NCH], F32, name=f"rank_col_{tag}")
            rank_ps = psum.tile([1, C], F32, name=f"rank_ps_{tag}", tag="ps_rank")
            gs = []
            for i in range(NCH):
                g = small.tile([P, C], F32, name=f"G_{tag}_{i}")
                nc.vector.tensor_scalar(
                    out=g[:],
                    in0=arow_b[:],
                    scalar1=acol[:, i : i + 1],
                    scalar2=None,
                    op0=OP.is_gt,
                    op1=OP.add,
                    accum_out=rank_col[:, i : i + 1],
                )
                gs.append(g)
            for i in range(NCH):
                nc.tensor.matmul(
                    out=rank_ps[:],
                    lhsT=ones_col[:],
                    rhs=gs[i][:],
                    start=(i == 0),
                    stop=(i == NCH - 1),
                )
            # colsum of G gives #{i : a[i] < a[j]}; rank[j] = (C-1) - colsum
            # (values assumed distinct, as in the reference's random normals)
            rank_row = small.tile([1, C], F32, name=f"rank_row_{tag}")
            nc.vector.tensor_scalar(
                out=rank_row[:], in0=rank_ps[:], scalar1=-1.0,
                scalar2=float(C - 1), op0=OP.mult, op1=OP.add,
            )

            # non-top masks (rank >= K); u8 copies because CopyPredicated
            # requires an integer mask
            z_row = small.tile([1, C], F32, name=f"z_row_{tag}")
            nc.vector.tensor_scalar(
                out=z_row[:], in0=rank_row[:], scalar1=K - 0.5, scalar2=None,
                op0=OP.is_gt,
            )
            z_row_m = small.tile([1, C], U8, name=f"z_row_m_{tag}")
            nc.vector.tensor_scalar(
                out=z_row_m[:], in0=rank_row[:], scalar1=K - 0.5, scalar2=None,
                op0=OP.is_gt,
            )
            z_col_m = small.tile([P, NCH], U8, name=f"z_col_m_{tag}")
            nc.vector.tensor_scalar(
                out=z_col_m[:], in0=rank_col[:], scalar1=K - 0.5, scalar2=None,
                op0=OP.is_gt,
            )

            # exclusive prefix sum of z along channel order
            pincl_row = small.tile([1, C], F32, name=f"pincl_row_{tag}")
            nc.vector.tensor_tensor_scan(
                out=pincl_row[:], data0=z_row[:], data1=zeros_row[:], initial=0.0,
                op0=OP.add, op1=OP.add,
            )
            pexcl_row = small.tile([1, C], F32, name=f"pexcl_row_{tag}")
            nc.vector.tensor_tensor(
                out=pexcl_row[:], in0=pincl_row[:], in1=z_row[:], op=OP.subtract
            )

            # masked prefix row (9999 on top channels), broadcast to partitions
            pm_row = small.tile([1, C], F32, name=f"pm_row_{tag}")
            nc.vector.select(
                out=pm_row[:], mask=z_row_m[:], on_true=pexcl_row[:],
                on_false=big_row[:],
            )
            pm_ps = psum.tile([P, C], F32, name=f"pm_ps_{tag}", tag="ps_pm")
            nc.tensor.matmul(
                out=pm_ps[:], lhsT=ones_row[:], rhs=pm_row[:], start=True, stop=True
            )
            pm_row_b = small.tile([P, C], F32, name=f"pm_row_b_{tag}")
            nc.vector.tensor_copy(pm_row_b[:], pm_ps[:])

            # prefix in column layout
            px_ps = psum.tile([P, NCH], F32, name=f"px_ps_{tag}", tag=f"ps_px_{tag}")
            for i in range(NCH):
                nc.tensor.matmul(
                    out=px_ps[:, i : i + 1],
                    lhsT=pexcl_row[0:1, i * P : (i + 1) * P],
                    rhs=ones_row[0:1, 0:1],
                    start=True,
                    stop=True,
                )
            px_col = small.tile([P, NCH], F32, name=f"px_col_{tag}")
            nc.vector.tensor_copy(px_col[:], px_ps[:])

            return z_col_m, px_col, pm_row_b

        z1_col, px1_col, pm1_row_b = bn_stats(a_raw1, "b1")
        z2_col, px2_col, pm2_row_b = bn_stats(a_raw2, "b2")

        def keep_dest(z_col, tag):
            """dest row for kept channels: c if in topk else OOB-skip."""
            df = small.tile([P, NCH], F32, name=f"dfk_{tag}")
            nc.vector.select(
                out=df[:], mask=z_col[:], on_true=oob_col[:], on_false=iota_col_f[:]
            )
            ds = []
            for k in range(NCH):
                dkk = small.tile([P, 1], I32, name=f"dk_{tag}_{k}")
                nc.vector.tensor_copy(dkk[:], df[:, k : k + 1])
                ds.append(dkk)
            return ds

        def exch_dest(z_col, px_col, other_pm_row_b, tag):
            """dest row in the OTHER output for non-top channels:
            nt_other[px[c]] (matched via is_equal against the masked
            other-side prefix row), OOB-skip for kept channels."""
            srcx_col = small.tile([P, NCH], F32, name=f"srcx_{tag}")
            for i in range(NCH):
                mt = small.tile([P, C], F32, name=f"mt_{tag}_{i}", tag="mt", bufs=2)
                nc.vector.scalar_tensor_tensor(
                    out=mt[:],
                    in0=other_pm_row_b[:],
                    scalar=px_col[:, i : i + 1],
                    in1=jrow_f[:],
                    op0=OP.is_equal,
                    op1=OP.mult,
                    accum_out=srcx_col[:, i : i + 1],
                )
            df = small.tile([P, NCH], F32, name=f"dfe_{tag}")
            nc.vector.select(
                out=df[:], mask=z_col[:], on_true=srcx_col[:], on_false=oob_col[:]
            )
            ds = []
            for k in range(NCH):
                dek = small.tile([P, 1], I32, name=f"de_{tag}_{k}")
                nc.vector.tensor_copy(dek[:], df[:, k : k + 1])
                ds.append(dek)
            return ds

        d_x1_y1 = keep_dest(z1_col, "x1y1")
        d_x2_y2 = keep_dest(z2_col, "x2y2")
        d_x1_y2 = exch_dest(z1_col, px1_col, pm2_row_b, "x1y2")
        d_x2_y1 = exch_dest(z2_col, px2_col, pm1_row_b, "x2y1")

        # ---- scatters: each input chunk goes once to y1 and once to y2,
        # with the rows belonging to the other output OOB-skipped.
        # Keep-scatters first (their dest tables are ready earliest);
        # alternate output tensors so same-tensor write ordering overlaps.
        def scat(y, d, k, src):
            nc.gpsimd.indirect_dma_start(
                out=y[:, :],
                out_offset=bass.IndirectOffsetOnAxis(ap=d[k][:, :], axis=0),
                in_=src[:],
                in_offset=None,
                bounds_check=C - 1,
                oob_is_err=False,
            )

        for k in range(NCH):
            scat(y1, d_x1_y1, k, xt1[k])
            scat(y2, d_x2_y2, k, xt2[k])
        for k in range(NCH):
            scat(y2, d_x1_y2, k, xt1[k])
            scat(y1, d_x2_y1, k, xt2[k])


def build_nc(compile=True):
    nc = bacc.Bacc(
        "TRN2",
        target_bir_lowering=False,
        debug=False,
        enable_asserts=False,
        num_devices=N_CORES,
    )
    with tile.TileContext(nc) as tc:
        _emit(tc)
    if compile:
        nc.compile()
    return nc


_NC = None


def _get_nc():
    global _NC
    if _NC is None:
        _NC = build_nc()
    return _NC


def kernel(x1, x2, bn1, bn2):
    global LAST_RESULTS
    x1 = np.ascontiguousarray(np.asarray(x1), dtype=np.float32)
    x2 = np.ascontiguousarray(np.asarray(x2), dtype=np.float32)
    bn1 = np.ascontiguousarray(np.asarray(bn1), dtype=np.float32)
    bn2 = np.ascontiguousarray(np.asarray(bn2), dtype=np.float32)
    assert x1.shape == (B, C, L) and x2.shape == (B, C, L)

    nc = _get_nc()
    in_maps = [
        {"x1": x1[i], "x2": x2[i], "bn1": bn1, "bn2": bn2}
        for i in range(N_CORES)
    ]
    res = run_bass_kernel_spmd(
        nc, in_maps, core_ids=list(range(N_CORES)), trace=TRACE
    )
    LAST_RESULTS = res
    y1 = np.stack([r["y1"] for r in res.results], axis=0)
    y2 = np.stack([r["y2"] for r in res.results], axis=0)
    return (y1, y2)
